# revision 17
# baseline (speedup 1.0000x reference)
"""Trainium2 Bass kernel for nn_CharTaggerBiLSTM, 8-core SPMD, 3 launches.

L1 char LSTM: data-parallel over batch (16 sentences/core). Transposed
   layout (features-on-partitions), f32r matmuls; emits the masked last
   hidden state per word -> DRAM.
L2 word LSTM: one direction per core (cores 0-3 forward, 4-7 backward),
   32 sentences/core so each weight stream serves twice the rows.
   Direction is data: backward cores receive the char outputs with the
   sentence axis reversed on host and their outputs are un-reversed.
   x-part/bias matmuls for step s+1 are issued during step s's
   elementwise work to keep PE fed.
L3 MLP + log_softmax: data-parallel (16 sentences/core), bf16 GEMMs.

Host does embedding gather, weight reshapes, the two reshard steps, and
reassembly.
"""

import sys
import functools
from contextlib import ExitStack

sys.path.insert(0, "/opt/trn_rl_repo")

import numpy as np
import ml_dtypes
from concourse import bacc, bass, mybir, tile, bass_utils

BF_NP = ml_dtypes.bfloat16

B, S, Lc = 128, 128, 20
AB, E = 100, 64
Hc, H, OUT = 256, 512, 50
NCORE = 8
BL = B // NCORE            # sentences per core in L1/L3
FP = mybir.dt.float32
FR = mybir.dt.float32r
BF = mybir.dt.bfloat16
G4 = 4 * Hc
WG = 4 * H

Sig = mybir.ActivationFunctionType.Sigmoid
TanhF = mybir.ActivationFunctionType.Tanh
ReluF = mybir.ActivationFunctionType.Relu
ExpF = mybir.ActivationFunctionType.Exp
LnF = mybir.ActivationFunctionType.Ln
IdentF = mybir.ActivationFunctionType.Identity


def build_l1(bl=BL):
    """Char LSTM, data-parallel; writes lastT [2,128,nl] to DRAM."""
    nl = bl * S
    nc = bacc.Bacc("TRN2", target_bir_lowering=False, debug=False,
                   num_devices=NCORE)
    d_eT = nc.dram_tensor("eT", [Lc, E, nl], FR, kind="ExternalInput")
    d_lenrep = nc.dram_tensor("lenrep", [128, nl], FP, kind="ExternalInput")
    d_cWxT = nc.dram_tensor("cWxT", [E, G4], FR, kind="ExternalInput")
    d_cWhT = nc.dram_tensor("cWhT", [2, 128, G4], FR, kind="ExternalInput")
    d_cbias = nc.dram_tensor("cbias", [128, G4 // 128], FP,
                             kind="ExternalInput")
    d_last = nc.dram_tensor("lastT", [2, 128, nl], FP, kind="ExternalOutput")

    CH = min(512, nl)
    NCH = (nl + CH - 1) // CH

    with tile.TileContext(nc) as tc:
        with ExitStack() as c1:
            cw = c1.enter_context(tc.tile_pool(name="cweights", bufs=1))
            cst = c1.enter_context(tc.tile_pool(name="cstate", bufs=1))
            ein = c1.enter_context(tc.tile_pool(name="ein", bufs=2))
            ctmp = c1.enter_context(tc.tile_pool(name="ctmp", bufs=2))
            cps = c1.enter_context(tc.tile_pool(name="cpsum", bufs=8,
                                                space="PSUM"))
            cWx = cw.tile([E, G4], FR, tag="cWx", name="cWx")
            cWh = cw.tile([128, 2, G4], FR, tag="cWh", name="cWh")
            cb = cw.tile([128, G4 // 128], FP, tag="cb", name="cb")
            lenr = cw.tile([128, nl], FP, tag="lenr", name="lenr")
            nc.sync.dma_start(cWx[:], d_cWxT.ap()[:])
            nc.sync.dma_start(cWh[:], d_cWhT.ap().rearrange("k p g -> p k g"))
            nc.sync.dma_start(cb[:], d_cbias.ap()[:])
            nc.sync.dma_start(lenr[:], d_lenrep.ap()[:])

            last = [cst.tile([128, nl], FP, tag=f"last{j}", name=f"last{j}")
                    for j in range(2)]
            hh = [cst.tile([128, nl], FR, tag=f"h{j}", name=f"h{j}")
                  for j in range(2)]
            cc = [cst.tile([128, nl], FP, tag=f"c{j}", name=f"c{j}")
                  for j in range(2)]
            for j in range(2):
                nc.vector.memset(cc[j][:], 0.0)
                nc.vector.memset(last[j][:], 0.0)

            for t in range(Lc):
                et = ein.tile([E, nl], FR, tag="et", name="et")
                nc.sync.dma_start(et[:], d_eT.ap()[t])
                for ci in range(NCH):
                    cs = slice(ci * CH, (ci + 1) * CH)
                    mask = ctmp.tile([128, CH], FP, tag="mask", name="mask")
                    nc.gpsimd.tensor_scalar(mask[:], lenr[:, cs], float(t),
                                            None, op0=mybir.AluOpType.is_gt)
                    ps = {}
                    for j in range(2):
                        for gi in range(4):
                            m = 2 * gi + j
                            p = cps.tile([128, CH], FP, tag="ps", name="ps")
                            ps[(j, gi)] = p
                            mm = [(cWx[:, m * 128:(m + 1) * 128], et[:, cs])]
                            if t > 0:
                                for k in range(2):
                                    mm.append((cWh[:, k, m * 128:(m + 1) * 128],
                                               hh[k][:, cs]))
                            for ki, (lhsT, rhs) in enumerate(mm):
                                nc.tensor.matmul(p[:], lhsT, rhs,
                                                 start=(ki == 0),
                                                 stop=(ki == len(mm) - 1))
                    for j in range(2):
                        bias = [cb[:, (2 * gi + j):(2 * gi + j) + 1]
                                for gi in range(4)]
                        i_s = ctmp.tile([128, CH], FP, tag="i_s", name="i_s")
                        f_s = ctmp.tile([128, CH], FP, tag="f_s", name="f_s")
                        g_t = ctmp.tile([128, CH], FP, tag="g_t", name="g_t")
                        o_s = ctmp.tile([128, CH], FP, tag="o_s", name="o_s")
                        nc.scalar.activation(i_s[:], ps[(j, 0)][:], Sig,
                                             bias=bias[0])
                        nc.scalar.activation(f_s[:], ps[(j, 1)][:], Sig,
                                             bias=bias[1])
                        nc.scalar.activation(g_t[:], ps[(j, 2)][:], TanhF,
                                             bias=bias[2])
                        nc.scalar.activation(o_s[:], ps[(j, 3)][:], Sig,
                                             bias=bias[3])
                        ig = ctmp.tile([128, CH], FP, tag="ig", name="ig")
                        nc.vector.tensor_mul(ig[:], i_s[:], g_t[:])
                        nc.vector.tensor_mul(cc[j][:, cs], f_s[:], cc[j][:, cs])
                        nc.vector.tensor_add(cc[j][:, cs], cc[j][:, cs], ig[:])
                        tc_t = ctmp.tile([128, CH], FP, tag="tc", name="tc")
                        nc.scalar.activation(tc_t[:], cc[j][:, cs], TanhF)
                        nc.vector.tensor_mul(hh[j][:, cs], o_s[:], tc_t[:])
                        dd = ctmp.tile([128, CH], FP, tag="dd", name="dd")
                        nc.gpsimd.tensor_sub(dd[:], hh[j][:, cs],
                                             last[j][:, cs])
                        nc.vector.tensor_mul(dd[:], dd[:], mask[:])
                        nc.vector.tensor_add(last[j][:, cs], last[j][:, cs],
                                             dd[:])
            for j in range(2):
                nc.sync.dma_start(d_last.ap()[j], last[j][:])
    nc.compile()
    return nc


def build_l2(bl2=2 * BL):
    """Word LSTM, one direction per core over bl2 sentences."""
    nl = bl2 * S
    nc = bacc.Bacc("TRN2", target_bir_lowering=False, debug=False,
                   num_devices=NCORE)
    d_last = nc.dram_tensor("lastT2", [2, 128, nl], FR, kind="ExternalInput")
    d_wW = nc.dram_tensor("wW", [6, 128, WG], FR, kind="ExternalInput")
    d_wb = nc.dram_tensor("wb", [1, WG], FR, kind="ExternalInput")
    d_ones = nc.dram_tensor("onesr", [1, 128], FR, kind="ExternalInput")
    d_eye = nc.dram_tensor("eye", [128, 128], FP, kind="ExternalInput")
    d_hs = nc.dram_tensor("hsTh", [4, 128, nl], BF, kind="ExternalOutput")

    with tile.TileContext(nc) as tc:
        with ExitStack() as c2:
            ww = c2.enter_context(tc.tile_pool(name="wweights", bufs=1))
            wst = c2.enter_context(tc.tile_pool(name="wstate", bufs=1))
            wtmp = c2.enter_context(tc.tile_pool(name="wtmp", bufs=2))
            wps = c2.enter_context(tc.tile_pool(name="wpsum", bufs=4,
                                                space="PSUM"))
            eye_sb = ww.tile([128, 128], FP, tag="eye", name="eye")
            nc.sync.dma_start(eye_sb[:], d_eye.ap()[:])
            ones = ww.tile([1, bl2], FR, tag="ones", name="ones")
            nc.sync.dma_start(ones[:], d_ones.ap()[:, 0:bl2])
            wbt = ww.tile([1, WG], FR, tag="wbt", name="wbt")
            nc.sync.dma_start(wbt[:], d_wb.ap()[:])
            wsb = ww.tile([128, 6, WG], FR, tag="wsb", name="wsb")
            nc.sync.dma_start(wsb[:], d_wW.ap().rearrange("k p g -> p k g"))

            lastT = [ww.tile([128, nl], FR, tag=f"lastT{j}", name=f"lastT{j}")
                     for j in range(2)]
            for j in range(2):
                nc.sync.dma_start(lastT[j][:], d_last.ap()[j])
            hsT = [wst.tile([128, nl], BF, tag=f"hsT{k}", name=f"hsT{k}")
                   for k in range(4)]
            cstate = wst.tile([bl2, H], FP, tag="wc", name="wc")
            nc.vector.memset(cstate[:], 0.0)
            ring = [wst.tile([128, bl2], FR, tag=f"rg{k}", name=f"rg{k}")
                    for k in range(4)]
            lastv = [lastT[j].rearrange("p (b s) -> p s b", s=S)
                     for j in range(2)]
            hsTv = [hsT[k].rearrange("p (b s) -> p s b", s=S)
                    for k in range(4)]

            # gates packed as two psum tiles: [i|f] and [g|o]; x-part +
            # bias of step s are issued one iteration early
            def xb_mms(s, gp):
                for gc in range(4):
                    t2, half = gp[gc // 2], (gc % 2) * H
                    dst = t2[:, half:half + H]
                    ns = slice(gc * H, (gc + 1) * H)
                    nc.tensor.matmul(dst, ones[:], wbt[:, ns],
                                     start=True, stop=False)
                    for k in range(2):
                        nc.tensor.matmul(dst, lastv[k][:, s, :],
                                         wsb[:, k, ns],
                                         start=False, stop=(s == 0 and k == 1))

            gps = [wps.tile([bl2, 2 * H], FP, tag="wps", name="wps")
                   for _ in range(2)]
            xb_mms(0, gps)
            for s in range(S):
                if s > 0:
                    for gc in range(4):
                        t2, half = gps[gc // 2], (gc % 2) * H
                        dst = t2[:, half:half + H]
                        ns = slice(gc * H, (gc + 1) * H)
                        for k in range(4):
                            nc.tensor.matmul(dst, ring[k][:],
                                             wsb[:, 2 + k, ns],
                                             start=False, stop=(k == 3))
                if_s = wtmp.tile([bl2, 2 * H], FP, tag="wif", name="wif")
                g_t = wtmp.tile([bl2, H], FP, tag="wg", name="wg")
                o_s = wtmp.tile([bl2, H], FP, tag="wo", name="wo")
                nc.scalar.activation(if_s[:], gps[0][:], Sig)
                nc.scalar.activation(g_t[:], gps[1][:, 0:H], TanhF)
                nc.scalar.activation(o_s[:], gps[1][:, H:2 * H], Sig)
                ig = wtmp.tile([bl2, H], FP, tag="wig", name="wig")
                nc.vector.tensor_mul(ig[:], if_s[:, 0:H], g_t[:])
                nc.vector.tensor_mul(cstate[:], if_s[:, H:2 * H], cstate[:])
                nc.vector.tensor_add(cstate[:], cstate[:], ig[:])
                tc_t = wtmp.tile([bl2, H], FP, tag="wtc", name="wtc")
                nc.scalar.activation(tc_t[:], cstate[:], TanhF)
                hrow = wtmp.tile([bl2, H], FP, tag="whr", name="whr")
                nc.vector.tensor_mul(hrow[:], o_s[:], tc_t[:])
                if s + 1 < S:
                    gps = [wps.tile([bl2, 2 * H], FP, tag="wps", name="wps")
                           for _ in range(2)]
                    xb_mms(s + 1, gps)
                tp4 = wps.tile([128, 4 * bl2], FP, tag="wps", name="tp4")
                for k in range(4):
                    tps_k = tp4[:, k * bl2:(k + 1) * bl2]
                    nc.tensor.transpose(tps_k, hrow[:, k * 128:(k + 1) * 128],
                                        eye_sb[0:bl2, 0:bl2])
                    nc.vector.tensor_copy(ring[k][:], tps_k)
                    nc.gpsimd.tensor_copy(hsTv[k][:, s, :], ring[k][:])
            for k in range(4):
                nc.sync.dma_start(d_hs.ap()[k], hsT[k][:])
    nc.compile()
    return nc


def build_l3(bl=BL):
    """MLP + log_softmax, data-parallel."""
    nl = bl * S
    nc = bacc.Bacc("TRN2", target_bir_lowering=False, debug=False,
                   num_devices=NCORE)
    d_hs = nc.dram_tensor("hsT8", [8, 128, nl], BF, kind="ExternalInput")
    d_W1T = nc.dram_tensor("W1T", [8, 128, 256], BF, kind="ExternalInput")
    d_b1 = nc.dram_tensor("b1m", [128, 2], FP, kind="ExternalInput")
    d_W2T = nc.dram_tensor("W2T", [2, 128, 256], BF, kind="ExternalInput")
    d_b2 = nc.dram_tensor("b2m", [128, 2], FP, kind="ExternalInput")
    d_W3T = nc.dram_tensor("W3T", [2, 128, OUT], BF, kind="ExternalInput")
    d_b3 = nc.dram_tensor("b3m", [OUT, 1], FP, kind="ExternalInput")
    d_eye = nc.dram_tensor("eye", [128, 128], FP, kind="ExternalInput")
    d_y = nc.dram_tensor("y", [nl, OUT], FP, kind="ExternalOutput")

    CH = min(512, nl)
    NCH = (nl + CH - 1) // CH

    with tile.TileContext(nc) as tc:
        with ExitStack() as c3:
            mw = c3.enter_context(tc.tile_pool(name="mweights", bufs=1))
            mact = c3.enter_context(tc.tile_pool(name="mact", bufs=1))
            mtmp = c3.enter_context(tc.tile_pool(name="mtmp", bufs=4))
            mps = c3.enter_context(tc.tile_pool(name="mpsum", bufs=2,
                                                space="PSUM"))
            sps = c3.enter_context(tc.tile_pool(name="spsum", bufs=2,
                                                space="PSUM"))
            eye_sb = mw.tile([128, 128], FP, tag="eye", name="eye")
            nc.sync.dma_start(eye_sb[:], d_eye.ap()[:])
            W1 = mw.tile([128, 8, 256], BF, tag="W1", name="W1")
            W2 = mw.tile([128, 2, 256], BF, tag="W2", name="W2")
            W3 = mw.tile([128, 2, OUT], BF, tag="W3", name="W3")
            b1 = mw.tile([128, 2], FP, tag="b1", name="b1")
            b2 = mw.tile([128, 2], FP, tag="b2", name="b2")
            b3 = mw.tile([OUT, 1], FP, tag="b3", name="b3")
            nc.sync.dma_start(W1[:], d_W1T.ap().rearrange("k p g -> p k g"))
            nc.sync.dma_start(W2[:], d_W2T.ap().rearrange("k p g -> p k g"))
            nc.sync.dma_start(W3[:], d_W3T.ap().rearrange("k p g -> p k g"))
            nc.sync.dma_start(b1[:], d_b1.ap()[:])
            nc.sync.dma_start(b2[:], d_b2.ap()[:])
            nc.sync.dma_start(b3[:], d_b3.ap()[:])
            hsT = [mw.tile([128, nl], BF, tag=f"hsT{k}", name=f"hsT{k}")
                   for k in range(8)]
            for k in range(8):
                nc.sync.dma_start(hsT[k][:], d_hs.ap()[k])
            h1 = [mact.tile([128, nl], BF, tag=f"h1{m}", name=f"h1{m}")
                  for m in range(2)]
            h2 = [mact.tile([128, nl], BF, tag=f"h2{m}", name=f"h2{m}")
                  for m in range(2)]
            for ci in range(NCH):
                cs = slice(ci * CH, (ci + 1) * CH)
                for m in range(2):
                    p = mps.tile([128, CH], FP, tag="mp1", name="mp1")
                    for k in range(8):
                        nc.tensor.matmul(
                            p[:], W1[:, k, m * 128:(m + 1) * 128],
                            hsT[k][:, cs], start=(k == 0), stop=(k == 7))
                    nc.scalar.activation(h1[m][:, cs], p[:], ReluF,
                                         bias=b1[:, m:m + 1])
            for ci in range(NCH):
                cs = slice(ci * CH, (ci + 1) * CH)
                for m in range(2):
                    p = mps.tile([128, CH], FP, tag="mp2", name="mp2")
                    for k in range(2):
                        nc.tensor.matmul(
                            p[:], W2[:, k, m * 128:(m + 1) * 128],
                            h1[k][:, cs], start=(k == 0), stop=(k == 1))
                    nc.scalar.activation(h2[m][:, cs], p[:], ReluF,
                                         bias=b2[:, m:m + 1])
            for pi in range(max(1, nl // 128)):
                pcount = min(128, nl - pi * 128)
                psl = slice(pi * 128, pi * 128 + pcount)
                lg = mps.tile([OUT, pcount], FP, tag="mp3", name="mp3")
                for k in range(2):
                    nc.tensor.matmul(lg[:], W3[:, k, :], h2[k][:, psl],
                                     start=(k == 0), stop=(k == 1))
                lgb = mtmp.tile([OUT, pcount], FP, tag="lgb", name="lgb")
                nc.scalar.activation(lgb[:], lg[:], IdentF, bias=b3[:, 0:1])
                lgr = sps.tile([pcount, OUT], FP, tag="lgr", name="lgr")
                nc.tensor.transpose(lgr[:], lgb[:], eye_sb[0:OUT, 0:OUT])
                nmx = mtmp.tile([pcount, 1], FP, tag="nmx", name="nmx")
                nc.vector.tensor_reduce(nmx[:], lgr[:],
                                        axis=mybir.AxisListType.X,
                                        op=mybir.AluOpType.max, negate=True)
                ex = mtmp.tile([pcount, OUT], FP, tag="ex", name="ex")
                sm = mtmp.tile([pcount, 1], FP, tag="sm", name="sm")
                nc.scalar.activation(ex[:], lgr[:], ExpF, bias=nmx[:],
                                     accum_out=sm[:])
                lsm = mtmp.tile([pcount, 1], FP, tag="lsm", name="lsm")
                nc.scalar.activation(lsm[:], sm[:], LnF)
                shift = mtmp.tile([pcount, 1], FP, tag="shift", name="shift")
                nc.vector.tensor_sub(shift[:], nmx[:], lsm[:])
                yt = mtmp.tile([pcount, OUT], FP, tag="yt", name="yt")
                nc.vector.tensor_scalar(yt[:], lgr[:], shift[:], None,
                                        op0=mybir.AluOpType.add)
                nc.sync.dma_start(d_y.ap()[psl, :], yt[:])
    nc.compile()
    return nc


@functools.lru_cache(maxsize=4)
def _modules(bl):
    return build_l1(bl), build_l2(2 * bl), build_l3(bl)


def _prep_shared(inputs):
    f32 = np.float32
    cWxT = np.ascontiguousarray(np.asarray(inputs["cW_ih"], f32).T)
    cWhT = np.ascontiguousarray(
        np.asarray(inputs["cW_hh"], f32).T).reshape(2, 128, G4)
    cbias = (np.asarray(inputs["cb_ih"], f32)
             + np.asarray(inputs["cb_hh"], f32))
    cbias_m = np.ascontiguousarray(cbias.reshape(G4 // 128, 128).T)
    wW, wb = [], []
    for pre in ("f", "b"):
        wih = np.asarray(inputs[pre + "W_ih"], f32)
        whh = np.asarray(inputs[pre + "W_hh"], f32)
        wW.append(np.ascontiguousarray(
            np.concatenate([wih.T, whh.T], 0)).reshape(6, 128, WG))
        wb.append((np.asarray(inputs[pre + "b_ih"], f32)
                   + np.asarray(inputs[pre + "b_hh"], f32)).reshape(1, WG))
    W1T = np.ascontiguousarray(
        np.asarray(inputs["W1"], f32).T.astype(BF_NP)).reshape(8, 128, 256)
    b1m = np.ascontiguousarray(np.asarray(inputs["b1"], f32).reshape(2, 128).T)
    W2T = np.ascontiguousarray(
        np.asarray(inputs["W2"], f32).T.astype(BF_NP)).reshape(2, 128, 256)
    b2m = np.ascontiguousarray(np.asarray(inputs["b2"], f32).reshape(2, 128).T)
    W3T = np.ascontiguousarray(
        np.asarray(inputs["W3"], f32).T.astype(BF_NP)).reshape(2, 128, OUT)
    b3m = np.ascontiguousarray(np.asarray(inputs["b3"], f32).reshape(OUT, 1))
    eye = np.eye(128, dtype=f32)
    onesr = np.ones((1, 128), f32)
    return dict(cWxT=cWxT, cWhT=cWhT, cbias=cbias_m, wW=wW, wb=wb, W1T=W1T,
                b1m=b1m, W2T=W2T, b2m=b2m, W3T=W3T, b3m=b3m, eye=eye,
                onesr=onesr)


def _l1_maps(inputs, sh, bl, ncores):
    x = np.asarray(inputs["x"])
    emb = np.asarray(inputs["emb"], np.float32)
    nl = bl * S
    maps = []
    for c in range(ncores):
        xc = x[c * bl:(c + 1) * bl].reshape(nl, Lc)
        lengths = (xc != 0).sum(axis=1).astype(np.float32)
        lenrep = np.ascontiguousarray(
            np.broadcast_to(lengths[None, :], (128, nl)))
        eT = np.ascontiguousarray(emb[xc].transpose(1, 2, 0))
        maps.append(dict(eT=eT, lenrep=lenrep, cWxT=sh["cWxT"],
                         cWhT=sh["cWhT"], cbias=sh["cbias"]))
    return maps


def _l2_maps(last_full, sh, bl2, ncores):
    # last_full: [2, 128, B*S] f32, columns ordered (b * S + s)
    maps = []
    half = ncores // 2
    for c in range(ncores):
        d = 0 if c < half else 1
        g = c % half
        lo, hi = g * bl2 * S, (g + 1) * bl2 * S
        lt = last_full[:, :, lo:hi]
        if d == 1:
            lt = lt.reshape(2, 128, bl2, S)[:, :, :, ::-1].reshape(
                2, 128, bl2 * S)
        maps.append(dict(lastT2=np.ascontiguousarray(lt), wW=sh["wW"][d],
                         wb=sh["wb"][d], onesr=sh["onesr"], eye=sh["eye"]))
    return maps


def _l3_maps(hs_f, hs_b, sh, bl, ncores):
    # hs_f/hs_b: [4, 128, B*S] bf16, full batch
    nl = bl * S
    maps = []
    for c in range(ncores):
        lo, hi = c * nl, (c + 1) * nl
        hs8 = np.concatenate([hs_f[:, :, lo:hi], hs_b[:, :, lo:hi]], axis=0)
        maps.append(dict(hsT8=np.ascontiguousarray(hs8), W1T=sh["W1T"],
                         b1m=sh["b1m"], W2T=sh["W2T"], b2m=sh["b2m"],
                         W3T=sh["W3T"], b3m=sh["b3m"], eye=sh["eye"]))
    return maps


def _pipeline(inputs, bl, ncores, run_l1, run_l2, run_l3):
    """Shared 3-launch pipeline; run_lX(in_maps) -> list of output dicts."""
    sh = _prep_shared(inputs)
    bl2 = 2 * bl
    half = ncores // 2

    r1 = run_l1(_l1_maps(inputs, sh, bl, ncores))
    last_full = np.concatenate([r1[c]["lastT"] for c in range(ncores)],
                               axis=2)

    r2 = run_l2(_l2_maps(last_full, sh, bl2, ncores))
    hs_f = np.concatenate([r2[g]["hsTh"] for g in range(half)], axis=2)
    hsb_parts = []
    for g in range(half):
        hb = np.asarray(r2[half + g]["hsTh"]).reshape(
            4, 128, bl2, S)[:, :, :, ::-1]
        hsb_parts.append(hb.reshape(4, 128, bl2 * S))
    hs_b = np.concatenate(hsb_parts, axis=2)

    r3 = run_l3(_l3_maps(hs_f, hs_b, sh, bl, ncores))
    out = np.empty((bl * ncores, S, OUT), np.float32)
    for c in range(ncores):
        out[c * bl:(c + 1) * bl] = np.asarray(r3[c]["y"]).reshape(bl, S, OUT)
    return out


def kernel(**inputs):
    l1, l2, l3 = _modules(BL)

    def runner(nc):
        def run(in_maps):
            res = bass_utils.run_bass_kernel_spmd(
                nc, in_maps, core_ids=list(range(NCORE)))
            return res.results
        return run

    return _pipeline(inputs, BL, NCORE, runner(l1), runner(l2), runner(l3))
